# revision 22
# baseline (speedup 1.0000x reference)
"""Cached multi-head attention decode step (1 query token, 32 heads, head 128,
KV len 8191+1, E=4096) on 8 NeuronCores, tensor-parallel over heads.

Sharding (hardcoded, core c of 8 owns 4 heads = 512 dims of the head axis):
  - Wq/Wk/Wv row-shards (output dims 512c:512c+512), passed pre-transposed as
    (4096, 512) so they serve directly as matmul moving operands.
  - KV cache column-shards: k as (512, 8191) transposed (head-dim major) for
    the QK^T matmul; v as (8191, 512) natural (key major) for the PV matmul.
  - The output projection reuses Wq (the reference reuses Wq_w): core c
    computes out[512c:512c+512] = attn_full @ Wq_c.T, with attn_full obtained
    by an on-device AllGather of the 8 per-core attention shards.
Per-core HBM traffic ~56 MiB; everything else is designed to hide under it.
"""

import math

import numpy as np

import concourse.bass as bass
import concourse.mybir as mybir
import concourse.tile as tile
from concourse import bacc
from concourse.bass_utils import run_bass_kernel_spmd

N_CORES = 8
EMBED = 4096
HEAD = 128
LOCAL_HEADS = 4                      # heads per core
SHARD = LOCAL_HEADS * HEAD           # 512
KV_LEN = 8191                        # cached keys; +1 new key computed on-device
KV_TOT = KV_LEN + 1                  # 8192
N_CHUNK = KV_TOT // 128              # 64 key chunks of 128
N_ICH = EMBED // 128                 # 32 contraction chunks for projections
ISQRT_D = 1.0 / math.sqrt(HEAD)
F32 = mybir.dt.float32


def _build_program(stop_after=None):
    nc = _emit_program(stop_after)
    nc.compile()
    return nc


def _emit_program(stop_after=None):
    nc = bacc.Bacc("TRN2", target_bir_lowering=False, debug=False,
                   num_devices=N_CORES)

    # ---- per-core external I/O (all float32) ----
    seqt = nc.dram_tensor("seqt", [128, N_ICH], F32, kind="ExternalInput")
    wqt = nc.dram_tensor("wqt", [EMBED, SHARD], F32, kind="ExternalInput")
    wkt = nc.dram_tensor("wkt", [EMBED, SHARD], F32, kind="ExternalInput")
    wvt = nc.dram_tensor("wvt", [EMBED, SHARD], F32, kind="ExternalInput")
    b3 = nc.dram_tensor("b3", [3, SHARD], F32, kind="ExternalInput")
    ktc = nc.dram_tensor("ktc", [SHARD, KV_LEN], F32, kind="ExternalInput")
    vc = nc.dram_tensor("vc", [KV_LEN, SHARD], F32, kind="ExternalInput")

    out_s = nc.dram_tensor("out_s", [1, SHARD], F32, kind="ExternalOutput")
    k_s = nc.dram_tensor("k_s", [1, SHARD], F32, kind="ExternalOutput")
    v_s = nc.dram_tensor("v_s", [1, SHARD], F32, kind="ExternalOutput")

    with tile.TileContext(nc) as tc:
        with tc.tile_pool(name="singles", bufs=1) as singles, \
             tc.tile_pool(name="wkv", bufs=3) as wkv, \
             tc.tile_pool(name="ktp", bufs=3) as ktp, \
             tc.tile_pool(name="vp", bufs=24) as vp, \
             tc.tile_pool(name="psA", bufs=1, space="PSUM") as psA, \
             tc.tile_pool(name="psB", bufs=2, space="PSUM") as psB, \
             tc.tile_pool(name="dram", bufs=1, space="DRAM") as dram:

            # ---------- resident SBUF ----------
            seqt_sb = singles.tile([128, N_ICH], F32)
            nc.sync.dma_start(out=seqt_sb, in_=seqt[:, :])

            bq_sb = singles.tile([1, SHARD], F32)
            bk_sb = singles.tile([1, SHARD], F32)
            bv_sb = singles.tile([1, SHARD], F32)
            nc.sync.dma_start(out=bq_sb, in_=b3[0:1, :])
            nc.sync.dma_start(out=bk_sb, in_=b3[1:2, :])
            nc.sync.dma_start(out=bv_sb, in_=b3[2:3, :])

            ones = singles.tile([128, 1], F32)
            nc.vector.memset(ones, 1.0)

            # Wq^T resident: (128, 32*512); reused by QKV proj and out proj.
            wq_sb = singles.tile([128, N_ICH * SHARD], F32)
            wq_sb3 = wq_sb.rearrange("p (i n) -> p i n", n=SHARD)
            for t in range(4):
                nc.sync.dma_start(
                    out=wq_sb3[:, t * 8:(t + 1) * 8, :],
                    in_=wqt[t * 1024:(t + 1) * 1024, :]
                        .rearrange("(i p) n -> p i n", p=128))

            probs_sb = singles.tile([128, LOCAL_HEADS * N_CHUNK], F32)
            sums_sb = singles.tile([128, LOCAL_HEADS], F32)
            recip_sb = singles.tile([LOCAL_HEADS, 1], F32)
            qt_sb = singles.tile([128, LOCAL_HEADS], F32)
            ktnew_sb = singles.tile([128, LOCAL_HEADS], F32)
            attnt_sb = singles.tile([128, N_ICH], F32)
            attn_sb = singles.tile([LOCAL_HEADS, SHARD], F32)
            stage_sb = singles.tile([1, SHARD], F32)

            # ---------- phase 1: q/k/v projections (M=1, N=512) ----------
            # bias folded into the accumulation group as a K=1 matmul with a
            # 1.0 weight, so the finished vectors live in PSUM directly.
            ps_q = psA.tile([1, SHARD], F32)
            for i in range(N_ICH):
                nc.tensor.matmul(ps_q, lhsT=seqt_sb[:, i:i + 1],
                                 rhs=wq_sb[:, i * SHARD:(i + 1) * SHARD],
                                 start=(i == 0), stop=False)
            nc.tensor.matmul(ps_q, lhsT=ones[0:1, 0:1], rhs=bq_sb,
                             start=False, stop=True)
            ps_k = psA.tile([1, SHARD], F32)
            ps_v = psA.tile([1, SHARD], F32)
            for i in range(N_ICH):
                wkv_t = wkv.tile([128, 2 * SHARD], F32, tag="wkv")
                nc.sync.dma_start(out=wkv_t[:, 0:SHARD],
                                  in_=wkt[i * 128:(i + 1) * 128, :])
                nc.sync.dma_start(out=wkv_t[:, SHARD:2 * SHARD],
                                  in_=wvt[i * 128:(i + 1) * 128, :])
                nc.tensor.matmul(ps_k, lhsT=seqt_sb[:, i:i + 1],
                                 rhs=wkv_t[:, 0:SHARD],
                                 start=(i == 0), stop=False)
                nc.tensor.matmul(ps_v, lhsT=seqt_sb[:, i:i + 1],
                                 rhs=wkv_t[:, SHARD:2 * SHARD],
                                 start=(i == 0), stop=False)
            nc.tensor.matmul(ps_k, lhsT=ones[0:1, 0:1], rhs=bk_sb,
                             start=False, stop=True)
            nc.tensor.matmul(ps_v, lhsT=ones[0:1, 0:1], rhs=bv_sb,
                             start=False, stop=True)

            # stage finished q/k/v rows to SBUF (DMA cannot read PSUM).
            # bk_sb/bv_sb are dead after the bias matmuls above; reuse them.
            nc.vector.tensor_copy(out=stage_sb, in_=ps_q)
            nc.vector.tensor_copy(out=bk_sb, in_=ps_k)
            nc.vector.tensor_copy(out=bv_sb, in_=ps_v)
            nc.sync.dma_start(out=k_s[:, :], in_=bk_sb)
            nc.sync.dma_start(out=v_s[:, :], in_=bv_sb)

            # transpose q,k rows to head-dim-on-partitions via a DRAM bounce
            qk_dram = dram.tile([2, SHARD], F32)
            nc.sync.dma_start(out=qk_dram[0:1, :], in_=stage_sb)
            nc.sync.dma_start(out=qk_dram[1:2, :], in_=bk_sb)
            nc.sync.dma_start(out=qt_sb,
                              in_=qk_dram[0:1, :].rearrange("1 (h p) -> p h", p=128))
            nc.sync.dma_start(out=ktnew_sb,
                              in_=qk_dram[1:2, :].rearrange("1 (h p) -> p h", p=128))


            if stop_after == "qkv":
                nc.vector.memset(attn_sb, 0.0)
                nc.sync.dma_start(out=out_s[:, :], in_=attn_sb[0:1, :])
                return nc
            # ---------- phase 2: scores + exp, per head ----------
            # kt tile (128, 4096) halves; scores chunk j -> PSUM col j.
            QCOLS = 2048
            NQ = KV_TOT // QCOLS  # 4 quarter-tiles per head
            for h in range(LOCAL_HEADS):
                ps_s = psB.tile([128, N_CHUNK], F32, tag="ps_s")
                for quarter in range(NQ):
                    kt_t = ktp.tile([128, QCOLS], F32, tag="kt")
                    c0 = quarter * QCOLS
                    ncols = min(QCOLS, KV_LEN - c0)  # 2047 in last quarter
                    nc.scalar.dma_start(
                        out=kt_t[:, 0:ncols],
                        in_=ktc[h * 128:(h + 1) * 128, c0:c0 + ncols])
                    if quarter == NQ - 1:
                        nc.vector.tensor_copy(out=kt_t[:, QCOLS - 1:QCOLS],
                                              in_=ktnew_sb[:, h:h + 1])
                    for j in range(QCOLS // 128):
                        jj = quarter * (QCOLS // 128) + j
                        nc.tensor.matmul(
                            ps_s[:, jj:jj + 1],
                            lhsT=kt_t[:, j * 128:(j + 1) * 128],
                            rhs=qt_sb[:, h:h + 1],
                            start=(jj == 0), stop=(jj == N_CHUNK - 1))
                nc.scalar.activation(
                    out=probs_sb[:, h * N_CHUNK:(h + 1) * N_CHUNK], in_=ps_s,
                    func=mybir.ActivationFunctionType.Exp,
                    scale=ISQRT_D,
                    accum_out=sums_sb[:, h:h + 1])

            if stop_after == "scores":
                nc.vector.memset(attn_sb, 0.0)
                nc.sync.dma_start(out=out_s[:, :], in_=attn_sb[0:1, :])
                return nc
            # ---------- phase 3: softmax denominators (all heads at once) ----
            ps_d = psA.tile([LOCAL_HEADS, 1], F32)
            nc.tensor.matmul(ps_d, lhsT=sums_sb, rhs=ones, start=True, stop=True)
            nc.vector.reciprocal(out=recip_sb, in_=ps_d)

            # ---------- phase 4: probs @ V (4 heads batched in M) ----------
            probs3 = probs_sb.rearrange("p (h j) -> p h j", h=LOCAL_HEADS)
            ps_attn = psA.tile([LOCAL_HEADS, SHARD], F32)
            for j in range(N_CHUNK):
                v_t = vp.tile([128, SHARD], F32, tag="v")
                if j < N_CHUNK - 1:
                    nc.gpsimd.dma_start(out=v_t,
                                        in_=vc[j * 128:(j + 1) * 128, :])
                else:
                    nc.gpsimd.dma_start(out=v_t[0:127, :],
                                        in_=vc[j * 128:KV_LEN, :])
                    nc.gpsimd.dma_start(out=v_t[127:128, :], in_=bv_sb)
                nc.tensor.matmul(ps_attn, lhsT=probs3[:, :, j],
                                 rhs=v_t,
                                 start=(j == 0), stop=(j == N_CHUNK - 1))
            nc.vector.tensor_scalar_mul(attn_sb, ps_attn, recip_sb)

            if stop_after == "av":
                nc.sync.dma_start(out=out_s[:, :], in_=attn_sb[0:1, :])
                return nc
            # ---------- phase 5: AllGather attention shards ----------
            cc_in = dram.tile([SHARD], F32)
            cc_out = dram.tile([EMBED], F32, addr_space="Shared")
            for h in range(LOCAL_HEADS):
                nc.sync.dma_start(
                    out=cc_in[h * HEAD:(h + 1) * HEAD],
                    in_=attn_sb[h:h + 1, h * HEAD:(h + 1) * HEAD])
            nc.gpsimd.collective_compute(
                "AllGather", mybir.AluOpType.bypass,
                replica_groups=[list(range(N_CORES))],
                ins=[cc_in.opt()], outs=[cc_out.opt()])
            nc.sync.dma_start(out=attnt_sb,
                              in_=cc_out[:].rearrange("(i p) -> p i", p=128))
            if stop_after == "cc":
                nc.sync.dma_start(out=out_s[:, :], in_=attn_sb[0:1, :])
                return nc

            # ---------- phase 6: output projection (reuses resident Wq^T) ---
            ps_o = psA.tile([1, SHARD], F32)
            for i in range(N_ICH):
                nc.tensor.matmul(ps_o, lhsT=attnt_sb[:, i:i + 1],
                                 rhs=wq_sb[:, i * SHARD:(i + 1) * SHARD],
                                 start=(i == 0), stop=False)
            nc.tensor.matmul(ps_o, lhsT=ones[0:1, 0:1], rhs=bq_sb,
                             start=False, stop=True)
            nc.vector.tensor_copy(out=stage_sb, in_=ps_o)
            nc.sync.dma_start(out=out_s[:, :], in_=stage_sb)

    return nc


_NC = None


def _get_nc():
    global _NC
    if _NC is None:
        _NC = _build_program()
    return _NC


def _make_in_maps(seq, k_cached, v_cached, Wq_w, Wq_b, Wk_w, Wk_b, Wv_w, Wv_b):
    f = lambda a: np.ascontiguousarray(np.asarray(a, dtype=np.float32))
    seq, k_cached, v_cached = f(seq), f(k_cached), f(v_cached)
    Wq_w, Wk_w, Wv_w = f(Wq_w), f(Wk_w), f(Wv_w)
    Wq_b, Wk_b, Wv_b = f(Wq_b), f(Wk_b), f(Wv_b)

    seqt = np.ascontiguousarray(seq.reshape(N_ICH, 128).T)      # (128, 32)
    ktc_full = np.ascontiguousarray(k_cached.T)                 # (4096, 8191)

    in_maps = []
    for c in range(N_CORES):
        sl = slice(c * SHARD, (c + 1) * SHARD)
        in_maps.append({
            "seqt": seqt,
            "wqt": np.ascontiguousarray(Wq_w[sl, :].T),
            "wkt": np.ascontiguousarray(Wk_w[sl, :].T),
            "wvt": np.ascontiguousarray(Wv_w[sl, :].T),
            "b3": np.ascontiguousarray(
                np.stack([Wq_b[sl], Wk_b[sl], Wv_b[sl]])),
            "ktc": ktc_full[sl, :],
            "vc": np.ascontiguousarray(v_cached[:, sl]),
        })
    return in_maps, (seq, k_cached, v_cached)


def _assemble(results, k_cached, v_cached):
    out = np.concatenate([results[c]["out_s"] for c in range(N_CORES)], axis=1)
    k_row = np.concatenate([results[c]["k_s"] for c in range(N_CORES)], axis=1)
    v_row = np.concatenate([results[c]["v_s"] for c in range(N_CORES)], axis=1)
    k_new = np.concatenate([k_cached, k_row], axis=0)
    v_new = np.concatenate([v_cached, v_row], axis=0)
    return out, k_new, v_new


def kernel(seq, k_cached, v_cached, Wq_w, Wq_b, Wk_w, Wk_b, Wv_w, Wv_b,
           _trace=False):
    nc = _get_nc()
    in_maps, (seq, k_cached, v_cached) = _make_in_maps(
        seq, k_cached, v_cached, Wq_w, Wq_b, Wk_w, Wk_b, Wv_w, Wv_b)
    res = run_bass_kernel_spmd(nc, in_maps, list(range(N_CORES)),
                               trace=_trace)
    outs = _assemble(res.results, k_cached, v_cached)
    if _trace:
        return outs, res
    return outs


# revision 24
# speedup vs baseline: 1.2504x; 1.2504x over previous
"""Cached multi-head attention decode step (1 query token, 32 heads, head 128,
KV len 8191+1, E=4096) on 8 NeuronCores, tensor-parallel over heads.

Sharding (hardcoded; core c of 8 owns 4 heads = 512 dims of the head axis):
  - Wq/Wk/Wv row-shards (output dims 512c:512c+512) passed pre-transposed as
    (4096, 512) moving operands; Wq^T kept resident in SBUF and reused for the
    output projection (the reference reuses Wq_w).
  - K/V cache column-shards in natural (key, dim) layout, streamed in
    (128, 4x512) chunk tiles.
  - AllGather of the 8 per-core attention shards feeds the output projection.

Engine split: projections and probs@V run on the PE (exact fp32, 4 cyc/row);
q.k scores run on the Vector engine as multiply+reduce against a
partition-broadcast q, so scores stream concurrently with PE work; softmax
denominators accumulate on the PE via a ones-column matmul sharing the AV
stationary. exp() per chunk on the Scalar engine. The per-chunk
score->exp->AV chain pipelines across DVE/ACT/PE.
"""

import math

import numpy as np

import concourse.bass as bass
import concourse.mybir as mybir
import concourse.tile as tile
from concourse import bacc
from concourse.bass_utils import run_bass_kernel_spmd

N_CORES = 8
EMBED = 4096
HEAD = 128
LOCAL_HEADS = 4                      # heads per core
SHARD = LOCAL_HEADS * HEAD           # 512
KV_LEN = 8191                        # cached keys; +1 new key computed on-device
KV_TOT = KV_LEN + 1                  # 8192
N_CHUNK = KV_TOT // 128              # 64 key chunks of 128
CPT = 4                              # chunks per streamed KV tile
N_KVT = N_CHUNK // CPT               # 16 KV tiles of (128, 4*512)
N_ICH = EMBED // 128                 # 32 contraction chunks for projections
ISQRT_D = 1.0 / math.sqrt(HEAD)
F32 = mybir.dt.float32


def _build_program(stop_after=None):
    nc = _emit_program(stop_after)
    nc.compile()
    return nc


def _emit_program(stop_after=None):
    nc = bacc.Bacc("TRN2", target_bir_lowering=False, debug=False,
                   num_devices=N_CORES)

    # ---- per-core external I/O (all float32) ----
    seqt = nc.dram_tensor("seqt", [128, N_ICH], F32, kind="ExternalInput")
    wqt = nc.dram_tensor("wqt", [EMBED, SHARD], F32, kind="ExternalInput")
    wkt = nc.dram_tensor("wkt", [EMBED, SHARD], F32, kind="ExternalInput")
    wvt = nc.dram_tensor("wvt", [EMBED, SHARD], F32, kind="ExternalInput")
    b3 = nc.dram_tensor("b3", [3, SHARD], F32, kind="ExternalInput")
    kc = nc.dram_tensor("kc", [KV_LEN, SHARD], F32, kind="ExternalInput")
    vc = nc.dram_tensor("vc", [KV_LEN, SHARD], F32, kind="ExternalInput")

    out_s = nc.dram_tensor("out_s", [1, SHARD], F32, kind="ExternalOutput")
    k_s = nc.dram_tensor("k_s", [1, SHARD], F32, kind="ExternalOutput")
    v_s = nc.dram_tensor("v_s", [1, SHARD], F32, kind="ExternalOutput")

    with tile.TileContext(nc) as tc:
        with tc.tile_pool(name="singles", bufs=1) as singles, \
             tc.tile_pool(name="wkv", bufs=4) as wkv, \
             tc.tile_pool(name="kp", bufs=4) as kp, \
             tc.tile_pool(name="vp", bufs=4) as vp, \
             tc.tile_pool(name="scr", bufs=2) as scr, \
             tc.tile_pool(name="psA", bufs=1, space="PSUM") as psA, \
             tc.tile_pool(name="dram", bufs=1, space="DRAM") as dram:

            # ---------- resident SBUF ----------
            seqt_sb = singles.tile([128, N_ICH], F32)
            nc.sync.dma_start(out=seqt_sb, in_=seqt[:, :])

            bq_sb = singles.tile([1, SHARD], F32)
            bk_sb = singles.tile([1, SHARD], F32)
            bv_sb = singles.tile([1, SHARD], F32)
            nc.sync.dma_start(out=bq_sb, in_=b3[0:1, :])
            nc.sync.dma_start(out=bk_sb, in_=b3[1:2, :])
            nc.sync.dma_start(out=bv_sb, in_=b3[2:3, :])

            ones = singles.tile([128, 1], F32)
            nc.vector.memset(ones, 1.0)

            # Wq^T resident: (128, 32*512); reused by QKV proj and out proj.
            wq_sb = singles.tile([128, N_ICH * SHARD], F32)
            wq_sb3 = wq_sb.rearrange("p (i n) -> p i n", n=SHARD)
            for t in range(4):
                nc.scalar.dma_start(
                    out=wq_sb3[:, t * 8:(t + 1) * 8, :],
                    in_=wqt[t * 1024:(t + 1) * 1024, :]
                        .rearrange("(i p) n -> p i n", p=128))

            q_bcast = singles.tile([128, SHARD], F32)
            scores_sb = singles.tile([128, N_CHUNK * LOCAL_HEADS], F32)
            probs_sb = singles.tile([128, N_CHUNK * LOCAL_HEADS], F32)
            recip_sb = singles.tile([LOCAL_HEADS, 1], F32)
            attnt_sb = singles.tile([128, N_ICH], F32)
            attn_sb = singles.tile([LOCAL_HEADS, SHARD], F32)
            stage_sb = singles.tile([1, SHARD], F32)

            # ---------- phase 1: q/k/v projections (M=1, N=512) ----------
            # bias folded into the accumulation group as a K=1 matmul.
            ps_q = psA.tile([1, SHARD], F32)
            for i in range(N_ICH):
                nc.tensor.matmul(ps_q, lhsT=seqt_sb[:, i:i + 1],
                                 rhs=wq_sb[:, i * SHARD:(i + 1) * SHARD],
                                 start=(i == 0), stop=False)
            nc.tensor.matmul(ps_q, lhsT=ones[0:1, 0:1], rhs=bq_sb,
                             start=False, stop=True)
            ps_k = psA.tile([1, SHARD], F32)
            ps_v = psA.tile([1, SHARD], F32)
            for i in range(N_ICH):
                wkv_t = wkv.tile([128, 2 * SHARD], F32, tag="wkv")
                nc.sync.dma_start(out=wkv_t[:, 0:SHARD],
                                  in_=wkt[i * 128:(i + 1) * 128, :])
                nc.sync.dma_start(out=wkv_t[:, SHARD:2 * SHARD],
                                  in_=wvt[i * 128:(i + 1) * 128, :])
                nc.tensor.matmul(ps_k, lhsT=seqt_sb[:, i:i + 1],
                                 rhs=wkv_t[:, 0:SHARD],
                                 start=(i == 0), stop=False)
                nc.tensor.matmul(ps_v, lhsT=seqt_sb[:, i:i + 1],
                                 rhs=wkv_t[:, SHARD:2 * SHARD],
                                 start=(i == 0), stop=False)
            nc.tensor.matmul(ps_k, lhsT=ones[0:1, 0:1], rhs=bk_sb,
                             start=False, stop=True)
            nc.tensor.matmul(ps_v, lhsT=ones[0:1, 0:1], rhs=bv_sb,
                             start=False, stop=True)

            # stage finished q/k/v rows in SBUF (DMA cannot read PSUM);
            # bk_sb/bv_sb are dead after the bias matmuls, reuse them.
            nc.vector.tensor_copy(out=stage_sb, in_=ps_q)
            nc.vector.tensor_copy(out=bk_sb, in_=ps_k)
            nc.vector.tensor_copy(out=bv_sb, in_=ps_v)
            nc.sync.dma_start(out=k_s[:, :], in_=bk_sb)
            nc.sync.dma_start(out=v_s[:, :], in_=bv_sb)

            # broadcast q across all 128 partitions via a DRAM bounce
            q_dram = dram.tile([1, SHARD], F32)
            nc.sync.dma_start(out=q_dram[:, :], in_=stage_sb)
            nc.sync.dma_start(out=q_bcast,
                              in_=q_dram[0:1, :].to_broadcast((128, SHARD)))

            if stop_after == "qkv":
                nc.vector.memset(attn_sb, 0.0)
                nc.sync.dma_start(out=out_s[:, :], in_=attn_sb[0:1, :])
                return nc

            # ---------- phase 2: streamed scores -> exp -> probs@V ----------
            # chunk j (128 keys): DVE computes q.k per head into scores cols
            # [4j:4j+4]; ACT exps them into probs; PE accumulates probs.T @ V
            # into (4,512) and probs.T @ ones into (4,1) denominators.
            ps_attn = psA.tile([LOCAL_HEADS, SHARD], F32)
            ps_d = psA.tile([LOCAL_HEADS, 1], F32)
            for t in range(N_KVT):
                k_t = kp.tile([128, CPT * SHARD], F32, tag="k")
                v_t = vp.tile([128, CPT * SHARD], F32, tag="v")
                r0 = t * CPT * 128
                if t < N_KVT - 1:
                    nc.scalar.dma_start(
                        out=k_t.rearrange("p (s n) -> p s n", n=SHARD),
                        in_=kc[r0:r0 + CPT * 128, :]
                            .rearrange("(s p) n -> p s n", p=128))
                    nc.gpsimd.dma_start(
                        out=v_t.rearrange("p (s n) -> p s n", n=SHARD),
                        in_=vc[r0:r0 + CPT * 128, :]
                            .rearrange("(s p) n -> p s n", p=128))
                else:
                    # chunks 60-62 full; chunk 63 has 127 cached keys + new row
                    nc.scalar.dma_start(
                        out=k_t[:, 0:(CPT - 1) * SHARD]
                            .rearrange("p (s n) -> p s n", n=SHARD),
                        in_=kc[r0:r0 + (CPT - 1) * 128, :]
                            .rearrange("(s p) n -> p s n", p=128))
                    nc.scalar.dma_start(
                        out=k_t[0:127, (CPT - 1) * SHARD:],
                        in_=kc[r0 + (CPT - 1) * 128:KV_LEN, :])
                    nc.sync.dma_start(out=k_t[127:128, (CPT - 1) * SHARD:],
                                      in_=bk_sb)
                    nc.gpsimd.dma_start(
                        out=v_t[:, 0:(CPT - 1) * SHARD]
                            .rearrange("p (s n) -> p s n", n=SHARD),
                        in_=vc[r0:r0 + (CPT - 1) * 128, :]
                            .rearrange("(s p) n -> p s n", p=128))
                    nc.gpsimd.dma_start(
                        out=v_t[0:127, (CPT - 1) * SHARD:],
                        in_=vc[r0 + (CPT - 1) * 128:KV_LEN, :])
                    nc.sync.dma_start(out=v_t[127:128, (CPT - 1) * SHARD:],
                                      in_=bv_sb)
                for s in range(CPT):
                    j = t * CPT + s
                    prod = scr.tile([128, SHARD], F32, tag="prod")
                    nc.vector.tensor_mul(
                        prod, k_t[:, s * SHARD:(s + 1) * SHARD], q_bcast)
                    nc.vector.tensor_reduce(
                        out=scores_sb[:, 4 * j:4 * j + 4],
                        in_=prod.rearrange("p (h d) -> p h d", d=HEAD),
                        axis=mybir.AxisListType.X,
                        op=mybir.AluOpType.add)
                    nc.scalar.activation(
                        out=probs_sb[:, 4 * j:4 * j + 4],
                        in_=scores_sb[:, 4 * j:4 * j + 4],
                        func=mybir.ActivationFunctionType.Exp,
                        scale=ISQRT_D)
                    nc.tensor.matmul(ps_attn,
                                     lhsT=probs_sb[:, 4 * j:4 * j + 4],
                                     rhs=v_t[:, s * SHARD:(s + 1) * SHARD],
                                     start=(j == 0), stop=(j == N_CHUNK - 1))
                    nc.tensor.matmul(ps_d,
                                     lhsT=probs_sb[:, 4 * j:4 * j + 4],
                                     rhs=ones,
                                     start=(j == 0), stop=(j == N_CHUNK - 1))

            nc.vector.reciprocal(out=recip_sb, in_=ps_d)
            nc.vector.tensor_scalar_mul(attn_sb, ps_attn, recip_sb)

            if stop_after == "av":
                nc.sync.dma_start(out=out_s[:, :], in_=attn_sb[0:1, :])
                return nc

            # ---------- phase 3: AllGather attention shards ----------
            cc_in = dram.tile([SHARD], F32)
            cc_out = dram.tile([EMBED], F32, addr_space="Shared")
            for h in range(LOCAL_HEADS):
                nc.sync.dma_start(
                    out=cc_in[h * HEAD:(h + 1) * HEAD],
                    in_=attn_sb[h:h + 1, h * HEAD:(h + 1) * HEAD])
            nc.gpsimd.collective_compute(
                "AllGather", mybir.AluOpType.bypass,
                replica_groups=[list(range(N_CORES))],
                ins=[cc_in.opt()], outs=[cc_out.opt()])
            nc.sync.dma_start(out=attnt_sb,
                              in_=cc_out[:].rearrange("(i p) -> p i", p=128))
            if stop_after == "cc":
                nc.sync.dma_start(out=out_s[:, :], in_=attn_sb[0:1, :])
                return nc

            # ---------- phase 4: output projection (resident Wq^T) ----------
            ps_o = psA.tile([1, SHARD], F32)
            for i in range(N_ICH):
                nc.tensor.matmul(ps_o, lhsT=attnt_sb[:, i:i + 1],
                                 rhs=wq_sb[:, i * SHARD:(i + 1) * SHARD],
                                 start=(i == 0), stop=False)
            nc.tensor.matmul(ps_o, lhsT=ones[0:1, 0:1], rhs=bq_sb,
                             start=False, stop=True)
            nc.vector.tensor_copy(out=stage_sb, in_=ps_o)
            nc.sync.dma_start(out=out_s[:, :], in_=stage_sb)

    return nc


_NC = None


def _get_nc():
    global _NC
    if _NC is None:
        _NC = _build_program()
    return _NC


def _make_in_maps(seq, k_cached, v_cached, Wq_w, Wq_b, Wk_w, Wk_b, Wv_w, Wv_b):
    f = lambda a: np.ascontiguousarray(np.asarray(a, dtype=np.float32))
    seq, k_cached, v_cached = f(seq), f(k_cached), f(v_cached)
    Wq_w, Wk_w, Wv_w = f(Wq_w), f(Wk_w), f(Wv_w)
    Wq_b, Wk_b, Wv_b = f(Wq_b), f(Wk_b), f(Wv_b)

    seqt = np.ascontiguousarray(seq.reshape(N_ICH, 128).T)      # (128, 32)

    in_maps = []
    for c in range(N_CORES):
        sl = slice(c * SHARD, (c + 1) * SHARD)
        in_maps.append({
            "seqt": seqt,
            "wqt": np.ascontiguousarray(Wq_w[sl, :].T),
            "wkt": np.ascontiguousarray(Wk_w[sl, :].T),
            "wvt": np.ascontiguousarray(Wv_w[sl, :].T),
            "b3": np.ascontiguousarray(
                np.stack([Wq_b[sl], Wk_b[sl], Wv_b[sl]])),
            "kc": np.ascontiguousarray(k_cached[:, sl]),
            "vc": np.ascontiguousarray(v_cached[:, sl]),
        })
    return in_maps, (seq, k_cached, v_cached)


def _assemble(results, k_cached, v_cached):
    out = np.concatenate([results[c]["out_s"] for c in range(N_CORES)], axis=1)
    k_row = np.concatenate([results[c]["k_s"] for c in range(N_CORES)], axis=1)
    v_row = np.concatenate([results[c]["v_s"] for c in range(N_CORES)], axis=1)
    k_new = np.concatenate([k_cached, k_row], axis=0)
    v_new = np.concatenate([v_cached, v_row], axis=0)
    return out, k_new, v_new


def kernel(seq, k_cached, v_cached, Wq_w, Wq_b, Wk_w, Wk_b, Wv_w, Wv_b,
           _trace=False):
    nc = _get_nc()
    in_maps, (seq, k_cached, v_cached) = _make_in_maps(
        seq, k_cached, v_cached, Wq_w, Wq_b, Wk_w, Wk_b, Wv_w, Wv_b)
    res = run_bass_kernel_spmd(nc, in_maps, list(range(N_CORES)),
                               trace=_trace)
    outs = _assemble(res.results, k_cached, v_cached)
    if _trace:
        return outs, res
    return outs


# revision 31
# speedup vs baseline: 1.2737x; 1.0186x over previous
"""Cached multi-head attention decode step (1 query token, 32 heads, head 128,
KV len 8191+1, E=4096) on 8 NeuronCores, tensor-parallel over heads.

Sharding (hardcoded; core c of 8 owns 4 heads = 512 dims of the head axis):
  - Wq/Wk/Wv row-shards (output dims 512c:512c+512) passed pre-transposed as
    (4096, 512) moving operands; Wq^T kept resident in SBUF and reused for the
    output projection (the reference reuses Wq_w).
  - K/V cache column-shards in natural (key, dim) layout, streamed in
    (128, 4x512) chunk tiles.
  - AllGather of the 8 per-core attention shards feeds the output projection.

Engine split: projections and probs@V run on the PE (exact fp32, 4 cyc/row);
q.k scores run on the Vector engine as multiply+reduce against a
partition-broadcast q, so scores stream concurrently with PE work; softmax
denominators accumulate on the PE via a ones-column matmul sharing the AV
stationary. exp() per chunk on the Scalar engine. The per-chunk
score->exp->AV chain pipelines across DVE/ACT/PE.
"""

import math

import numpy as np

import concourse.bass as bass
import concourse.mybir as mybir
import concourse.tile as tile
from concourse import bacc
from concourse.bass_utils import run_bass_kernel_spmd

N_CORES = 8
EMBED = 4096
HEAD = 128
LOCAL_HEADS = 4                      # heads per core
SHARD = LOCAL_HEADS * HEAD           # 512
KV_LEN = 8191                        # cached keys; +1 new key computed on-device
KV_TOT = KV_LEN + 1                  # 8192
N_CHUNK = KV_TOT // 128              # 64 key chunks of 128
CPT = 4                              # chunks per streamed KV tile
N_KVT = N_CHUNK // CPT               # 16 KV tiles of (128, 4*512)
N_ICH = EMBED // 128                 # 32 contraction chunks for projections
ISQRT_D = 1.0 / math.sqrt(HEAD)
F32 = mybir.dt.float32


def _build_program(stop_after=None):
    nc = _emit_program(stop_after)
    nc.compile()
    return nc


def _emit_program(stop_after=None):
    nc = bacc.Bacc("TRN2", target_bir_lowering=False, debug=False,
                   num_devices=N_CORES)

    # ---- per-core external I/O (all float32) ----
    seqt = nc.dram_tensor("seqt", [128, N_ICH], F32, kind="ExternalInput")
    wqt = nc.dram_tensor("wqt", [EMBED, SHARD], F32, kind="ExternalInput")
    wkt = nc.dram_tensor("wkt", [EMBED, SHARD], F32, kind="ExternalInput")
    wvt = nc.dram_tensor("wvt", [EMBED, SHARD], F32, kind="ExternalInput")
    b3 = nc.dram_tensor("b3", [3, SHARD], F32, kind="ExternalInput")
    kc = nc.dram_tensor("kc", [KV_LEN, SHARD], F32, kind="ExternalInput")
    vc = nc.dram_tensor("vc", [KV_LEN, SHARD], F32, kind="ExternalInput")

    out_s = nc.dram_tensor("out_s", [1, SHARD], F32, kind="ExternalOutput")
    k_s = nc.dram_tensor("k_s", [1, SHARD], F32, kind="ExternalOutput")
    v_s = nc.dram_tensor("v_s", [1, SHARD], F32, kind="ExternalOutput")

    with tile.TileContext(nc) as tc:
        with tc.tile_pool(name="singles", bufs=1) as singles, \
             tc.tile_pool(name="wkv", bufs=4) as wkv, \
             tc.tile_pool(name="kp", bufs=4) as kp, \
             tc.tile_pool(name="vp", bufs=4) as vp, \
             tc.tile_pool(name="scr", bufs=2) as scr, \
             tc.tile_pool(name="psA", bufs=1, space="PSUM") as psA, \
             tc.tile_pool(name="dram", bufs=1, space="DRAM") as dram:

            # ---------- resident SBUF ----------
            seqt_sb = singles.tile([128, N_ICH], F32)
            nc.sync.dma_start(out=seqt_sb, in_=seqt[:, :])

            bq_sb = singles.tile([1, SHARD], F32)
            bk_sb = singles.tile([1, SHARD], F32)
            bv_sb = singles.tile([1, SHARD], F32)
            nc.sync.dma_start(out=bq_sb, in_=b3[0:1, :])
            nc.sync.dma_start(out=bk_sb, in_=b3[1:2, :])
            nc.sync.dma_start(out=bv_sb, in_=b3[2:3, :])

            ones = singles.tile([128, 1], F32)
            nc.vector.memset(ones, 1.0)

            # Wq^T resident: (128, 32*512); reused by QKV proj and out proj.
            # 8 DMAs so the first q matmuls can start early.
            wq_sb = singles.tile([128, N_ICH * SHARD], F32)
            wq_sb3 = wq_sb.rearrange("p (i n) -> p i n", n=SHARD)
            for t in range(8):
                nc.scalar.dma_start(
                    out=wq_sb3[:, t * 4:(t + 1) * 4, :],
                    in_=wqt[t * 512:(t + 1) * 512, :]
                        .rearrange("(i p) n -> p i n", p=128))

            q_bcast = singles.tile([128, CPT * SHARD], F32)
            scores_sb = singles.tile([128, N_CHUNK * LOCAL_HEADS], F32)
            probs_sb = singles.tile([128, N_CHUNK * LOCAL_HEADS], F32)
            recip_sb = singles.tile([LOCAL_HEADS, 1], F32)
            attnt_sb = singles.tile([128, N_ICH], F32)
            attn_sb = singles.tile([LOCAL_HEADS, SHARD], F32)
            stage_sb = singles.tile([1, SHARD], F32)

            # ---------- phase 1: q/k/v projections (M=1, N=512) ----------
            # bias folded into the accumulation group as a K=1 matmul.
            ps_q = psA.tile([1, SHARD], F32)
            for i in range(N_ICH):
                nc.tensor.matmul(ps_q, lhsT=seqt_sb[:, i:i + 1],
                                 rhs=wq_sb[:, i * SHARD:(i + 1) * SHARD],
                                 start=(i == 0), stop=False)
            nc.tensor.matmul(ps_q, lhsT=ones[0:1, 0:1], rhs=bq_sb,
                             start=False, stop=True)
            ps_k = psA.tile([1, SHARD], F32)
            ps_v = psA.tile([1, SHARD], F32)
            for i in range(N_ICH):
                wk_t = wkv.tile([128, SHARD], F32, tag="wk")
                wv_t = wkv.tile([128, SHARD], F32, tag="wv")
                nc.sync.dma_start(out=wk_t,
                                  in_=wkt[i * 128:(i + 1) * 128, :])
                nc.gpsimd.dma_start(out=wv_t,
                                    in_=wvt[i * 128:(i + 1) * 128, :])
                nc.tensor.matmul(ps_k, lhsT=seqt_sb[:, i:i + 1],
                                 rhs=wk_t,
                                 start=(i == 0), stop=False)
                nc.tensor.matmul(ps_v, lhsT=seqt_sb[:, i:i + 1],
                                 rhs=wv_t,
                                 start=(i == 0), stop=False)
            nc.tensor.matmul(ps_k, lhsT=ones[0:1, 0:1], rhs=bk_sb,
                             start=False, stop=True)
            nc.tensor.matmul(ps_v, lhsT=ones[0:1, 0:1], rhs=bv_sb,
                             start=False, stop=True)

            # stage finished q/k/v rows in SBUF (DMA cannot read PSUM);
            # bk_sb/bv_sb are dead after the bias matmuls, reuse them.
            nc.vector.tensor_copy(out=stage_sb, in_=ps_q)
            nc.vector.tensor_copy(out=bk_sb, in_=ps_k)
            nc.vector.tensor_copy(out=bv_sb, in_=ps_v)
            nc.sync.dma_start(out=k_s[:, :], in_=bk_sb)
            nc.sync.dma_start(out=v_s[:, :], in_=bv_sb)

            # broadcast q across all 128 partitions via a DRAM bounce.
            # On the scalar queue (right after the wq loads) so it isn't
            # stuck behind the weight streams.
            q_dram = dram.tile([1, SHARD], F32)
            nc.scalar.dma_start(out=q_dram[:, :], in_=stage_sb)
            for s in range(CPT):
                nc.scalar.dma_start(
                    out=q_bcast[:, s * SHARD:(s + 1) * SHARD],
                    in_=q_dram[0:1, :].to_broadcast((128, SHARD)))

            if stop_after == "qkv":
                nc.vector.memset(attn_sb, 0.0)
                nc.sync.dma_start(out=out_s[:, :], in_=attn_sb[0:1, :])
                return nc

            # ---------- phase 2: streamed scores -> exp -> probs@V ----------
            # chunk j (128 keys): DVE computes q.k per head into scores cols
            # [4j:4j+4]; ACT exps them into probs; PE accumulates probs.T @ V
            # into (4,512) and probs.T @ ones into (4,1) denominators.
            ps_attn = psA.tile([LOCAL_HEADS, SHARD], F32)
            ps_d = psA.tile([LOCAL_HEADS, 1], F32)
            for t in range(N_KVT):
                k_t = kp.tile([128, CPT * SHARD], F32, tag="k")
                v_t = vp.tile([128, CPT * SHARD], F32, tag="v")
                r0 = t * CPT * 128
                if t < N_KVT - 1:
                    nc.scalar.dma_start(
                        out=k_t.rearrange("p (s n) -> p s n", n=SHARD),
                        in_=kc[r0:r0 + CPT * 128, :]
                            .rearrange("(s p) n -> p s n", p=128))
                    nc.gpsimd.dma_start(
                        out=v_t.rearrange("p (s n) -> p s n", n=SHARD),
                        in_=vc[r0:r0 + CPT * 128, :]
                            .rearrange("(s p) n -> p s n", p=128))
                else:
                    # chunks 60-62 full; chunk 63 has 127 cached keys + new row
                    nc.scalar.dma_start(
                        out=k_t[:, 0:(CPT - 1) * SHARD]
                            .rearrange("p (s n) -> p s n", n=SHARD),
                        in_=kc[r0:r0 + (CPT - 1) * 128, :]
                            .rearrange("(s p) n -> p s n", p=128))
                    nc.scalar.dma_start(
                        out=k_t[0:127, (CPT - 1) * SHARD:],
                        in_=kc[r0 + (CPT - 1) * 128:KV_LEN, :])
                    nc.sync.dma_start(out=k_t[127:128, (CPT - 1) * SHARD:],
                                      in_=bk_sb)
                    nc.gpsimd.dma_start(
                        out=v_t[:, 0:(CPT - 1) * SHARD]
                            .rearrange("p (s n) -> p s n", n=SHARD),
                        in_=vc[r0:r0 + (CPT - 1) * 128, :]
                            .rearrange("(s p) n -> p s n", p=128))
                    nc.gpsimd.dma_start(
                        out=v_t[0:127, (CPT - 1) * SHARD:],
                        in_=vc[r0 + (CPT - 1) * 128:KV_LEN, :])
                    nc.sync.dma_start(out=v_t[127:128, (CPT - 1) * SHARD:],
                                      in_=bv_sb)
                # one wide multiply + one segmented reduce + one exp per tile
                prod = scr.tile([128, CPT * SHARD], F32, tag="prod")
                nc.vector.tensor_mul(prod, k_t, q_bcast)
                nc.vector.tensor_reduce(
                    out=scores_sb[:, CPT * LOCAL_HEADS * t:
                                  CPT * LOCAL_HEADS * (t + 1)],
                    in_=prod.rearrange("p (c d) -> p c d", d=HEAD),
                    axis=mybir.AxisListType.X,
                    op=mybir.AluOpType.add)
                nc.scalar.activation(
                    out=probs_sb[:, CPT * LOCAL_HEADS * t:
                                 CPT * LOCAL_HEADS * (t + 1)],
                    in_=scores_sb[:, CPT * LOCAL_HEADS * t:
                                  CPT * LOCAL_HEADS * (t + 1)],
                    func=mybir.ActivationFunctionType.Exp,
                    scale=ISQRT_D)
                for s in range(CPT):
                    j = t * CPT + s
                    nc.tensor.matmul(ps_attn,
                                     lhsT=probs_sb[:, 4 * j:4 * j + 4],
                                     rhs=v_t[:, s * SHARD:(s + 1) * SHARD],
                                     start=(j == 0), stop=(j == N_CHUNK - 1))
                    nc.tensor.matmul(ps_d,
                                     lhsT=probs_sb[:, 4 * j:4 * j + 4],
                                     rhs=ones,
                                     start=(j == 0), stop=(j == N_CHUNK - 1))

            nc.vector.reciprocal(out=recip_sb, in_=ps_d)
            nc.vector.tensor_scalar_mul(attn_sb, ps_attn, recip_sb)

            if stop_after == "av":
                nc.sync.dma_start(out=out_s[:, :], in_=attn_sb[0:1, :])
                return nc

            # ---------- phase 3: AllGather attention shards ----------
            cc_in = dram.tile([SHARD], F32)
            cc_out = dram.tile([EMBED], F32, addr_space="Shared")
            for h in range(LOCAL_HEADS):
                nc.sync.dma_start(
                    out=cc_in[h * HEAD:(h + 1) * HEAD],
                    in_=attn_sb[h:h + 1, h * HEAD:(h + 1) * HEAD])
            nc.gpsimd.collective_compute(
                "AllGather", mybir.AluOpType.bypass,
                replica_groups=[list(range(N_CORES))],
                ins=[cc_in.opt()], outs=[cc_out.opt()])
            nc.sync.dma_start(out=attnt_sb,
                              in_=cc_out[:].rearrange("(i p) -> p i", p=128))
            if stop_after == "cc":
                nc.sync.dma_start(out=out_s[:, :], in_=attn_sb[0:1, :])
                return nc

            # ---------- phase 4: output projection (resident Wq^T) ----------
            ps_o = psA.tile([1, SHARD], F32)
            for i in range(N_ICH):
                nc.tensor.matmul(ps_o, lhsT=attnt_sb[:, i:i + 1],
                                 rhs=wq_sb[:, i * SHARD:(i + 1) * SHARD],
                                 start=(i == 0), stop=False)
            nc.tensor.matmul(ps_o, lhsT=ones[0:1, 0:1], rhs=bq_sb,
                             start=False, stop=True)
            nc.vector.tensor_copy(out=stage_sb, in_=ps_o)
            nc.sync.dma_start(out=out_s[:, :], in_=stage_sb)

    return nc


_NC = None


def _get_nc():
    global _NC
    if _NC is None:
        _NC = _build_program()
    return _NC


def _make_in_maps(seq, k_cached, v_cached, Wq_w, Wq_b, Wk_w, Wk_b, Wv_w, Wv_b):
    f = lambda a: np.ascontiguousarray(np.asarray(a, dtype=np.float32))
    seq, k_cached, v_cached = f(seq), f(k_cached), f(v_cached)
    Wq_w, Wk_w, Wv_w = f(Wq_w), f(Wk_w), f(Wv_w)
    Wq_b, Wk_b, Wv_b = f(Wq_b), f(Wk_b), f(Wv_b)

    seqt = np.ascontiguousarray(seq.reshape(N_ICH, 128).T)      # (128, 32)

    in_maps = []
    for c in range(N_CORES):
        sl = slice(c * SHARD, (c + 1) * SHARD)
        in_maps.append({
            "seqt": seqt,
            "wqt": np.ascontiguousarray(Wq_w[sl, :].T),
            "wkt": np.ascontiguousarray(Wk_w[sl, :].T),
            "wvt": np.ascontiguousarray(Wv_w[sl, :].T),
            "b3": np.ascontiguousarray(
                np.stack([Wq_b[sl], Wk_b[sl], Wv_b[sl]])),
            "kc": np.ascontiguousarray(k_cached[:, sl]),
            "vc": np.ascontiguousarray(v_cached[:, sl]),
        })
    return in_maps, (seq, k_cached, v_cached)


def _assemble(results, k_cached, v_cached):
    out = np.concatenate([results[c]["out_s"] for c in range(N_CORES)], axis=1)
    k_row = np.concatenate([results[c]["k_s"] for c in range(N_CORES)], axis=1)
    v_row = np.concatenate([results[c]["v_s"] for c in range(N_CORES)], axis=1)
    k_new = np.concatenate([k_cached, k_row], axis=0)
    v_new = np.concatenate([v_cached, v_row], axis=0)
    return out, k_new, v_new


def kernel(seq, k_cached, v_cached, Wq_w, Wq_b, Wk_w, Wk_b, Wv_w, Wv_b,
           _trace=False):
    nc = _get_nc()
    in_maps, (seq, k_cached, v_cached) = _make_in_maps(
        seq, k_cached, v_cached, Wq_w, Wq_b, Wk_w, Wk_b, Wv_w, Wv_b)
    res = run_bass_kernel_spmd(nc, in_maps, list(range(N_CORES)),
                               trace=_trace)
    outs = _assemble(res.results, k_cached, v_cached)
    if _trace:
        return outs, res
    return outs


# revision 33
# speedup vs baseline: 1.3480x; 1.0583x over previous
"""Cached multi-head attention decode step (1 query token, 32 heads, head 128,
KV len 8191+1, E=4096) on 8 NeuronCores, tensor-parallel over heads.

Sharding (hardcoded; core c of 8 owns 4 heads = 512 dims of the head axis):
  - Wq/Wk/Wv row-shards (output dims 512c:512c+512) passed pre-transposed as
    (4096, 512) moving operands; Wq^T kept resident in SBUF and reused for the
    output projection (the reference reuses Wq_w).
  - K/V cache column-shards in natural (key, dim) layout, streamed in
    (128, 4x512) chunk tiles.
  - AllGather of the 8 per-core attention shards feeds the output projection.

Engine split: projections and probs@V run on the PE (exact fp32, 4 cyc/row);
q.k scores run on the Vector engine as multiply+reduce against a
partition-broadcast q, so scores stream concurrently with PE work; softmax
denominators accumulate on the PE via a ones-column matmul sharing the AV
stationary. exp() per chunk on the Scalar engine. The per-chunk
score->exp->AV chain pipelines across DVE/ACT/PE.
"""

import math

import numpy as np

import concourse.bass as bass
import concourse.mybir as mybir
import concourse.tile as tile
from concourse import bacc
from concourse.bass_utils import run_bass_kernel_spmd

N_CORES = 8
EMBED = 4096
HEAD = 128
LOCAL_HEADS = 4                      # heads per core
SHARD = LOCAL_HEADS * HEAD           # 512
KV_LEN = 8191                        # cached keys; +1 new key computed on-device
KV_TOT = KV_LEN + 1                  # 8192
N_CHUNK = KV_TOT // 128              # 64 key chunks of 128
CPT = 4                              # chunks per streamed KV tile
N_KVT = N_CHUNK // CPT               # 16 KV tiles of (128, 4*512)
N_ICH = EMBED // 128                 # 32 contraction chunks for projections
ISQRT_D = 1.0 / math.sqrt(HEAD)
F32 = mybir.dt.float32


def _build_program(stop_after=None):
    nc = _emit_program(stop_after)
    nc.compile()
    return nc


def _emit_program(stop_after=None):
    nc = bacc.Bacc("TRN2", target_bir_lowering=False, debug=False,
                   num_devices=N_CORES)

    # ---- per-core external I/O (all float32) ----
    seqt = nc.dram_tensor("seqt", [128, N_ICH], F32, kind="ExternalInput")
    wqt = nc.dram_tensor("wqt", [EMBED, SHARD], F32, kind="ExternalInput")
    wkt = nc.dram_tensor("wkt", [EMBED, SHARD], F32, kind="ExternalInput")
    wvt = nc.dram_tensor("wvt", [EMBED, SHARD], F32, kind="ExternalInput")
    b3 = nc.dram_tensor("b3", [3, SHARD], F32, kind="ExternalInput")
    kc = nc.dram_tensor("kc", [KV_LEN, SHARD], F32, kind="ExternalInput")
    vc = nc.dram_tensor("vc", [KV_LEN, SHARD], F32, kind="ExternalInput")

    out_s = nc.dram_tensor("out_s", [1, SHARD], F32, kind="ExternalOutput")
    k_s = nc.dram_tensor("k_s", [1, SHARD], F32, kind="ExternalOutput")
    v_s = nc.dram_tensor("v_s", [1, SHARD], F32, kind="ExternalOutput")

    with tile.TileContext(nc) as tc:
        with tc.tile_pool(name="singles", bufs=1) as singles, \
             tc.tile_pool(name="wkv", bufs=4) as wkv, \
             tc.tile_pool(name="kp", bufs=4) as kp, \
             tc.tile_pool(name="vp", bufs=4) as vp, \
             tc.tile_pool(name="scr", bufs=2) as scr, \
             tc.tile_pool(name="psA", bufs=1, space="PSUM") as psA, \
             tc.tile_pool(name="dram", bufs=1, space="DRAM") as dram:

            # ---------- resident SBUF ----------
            seqt_sb = singles.tile([128, N_ICH], F32)
            nc.sync.dma_start(out=seqt_sb, in_=seqt[:, :])

            bq_sb = singles.tile([1, SHARD], F32)
            bk_sb = singles.tile([1, SHARD], F32)
            bv_sb = singles.tile([1, SHARD], F32)
            nc.sync.dma_start(out=bq_sb, in_=b3[0:1, :])
            nc.sync.dma_start(out=bk_sb, in_=b3[1:2, :])
            nc.sync.dma_start(out=bv_sb, in_=b3[2:3, :])

            ones = singles.tile([128, 1], F32)
            nc.vector.memset(ones, 1.0)

            # Wq^T resident: (128, 32*512); reused by QKV proj and out proj.
            # 8 DMAs split across the sync/gpsimd queues so the first q
            # matmuls start early and the scalar queue stays free for the
            # q-broadcast + K-cache stream.
            wq_sb = singles.tile([128, N_ICH * SHARD], F32)
            wq_sb3 = wq_sb.rearrange("p (i n) -> p i n", n=SHARD)
            for t in range(8):
                eng = nc.sync if t % 2 == 0 else nc.gpsimd
                eng.dma_start(
                    out=wq_sb3[:, t * 4:(t + 1) * 4, :],
                    in_=wqt[t * 512:(t + 1) * 512, :]
                        .rearrange("(i p) n -> p i n", p=128))

            q_bcast = singles.tile([128, CPT * SHARD], F32)
            scores_sb = singles.tile([128, N_CHUNK * LOCAL_HEADS], F32)
            probs_sb = singles.tile([128, N_CHUNK * LOCAL_HEADS], F32)
            recip_sb = singles.tile([LOCAL_HEADS, 1], F32)
            attnt_sb = singles.tile([128, N_ICH], F32)
            attn_sb = singles.tile([LOCAL_HEADS, SHARD], F32)
            stage_sb = singles.tile([1, SHARD], F32)

            # ---------- phase 1: q/k/v projections (M=1, N=512) ----------
            # bias folded into the accumulation group as a K=1 matmul.
            ps_q = psA.tile([1, SHARD], F32)
            for i in range(N_ICH):
                nc.tensor.matmul(ps_q, lhsT=seqt_sb[:, i:i + 1],
                                 rhs=wq_sb[:, i * SHARD:(i + 1) * SHARD],
                                 start=(i == 0), stop=False)
            nc.tensor.matmul(ps_q, lhsT=ones[0:1, 0:1], rhs=bq_sb,
                             start=False, stop=True)
            ps_k = psA.tile([1, SHARD], F32)
            ps_v = psA.tile([1, SHARD], F32)
            for i2 in range(N_ICH // 2):
                wk_t = wkv.tile([128, 2 * SHARD], F32, tag="wk")
                wv_t = wkv.tile([128, 2 * SHARD], F32, tag="wv")
                nc.sync.dma_start(
                    out=wk_t.rearrange("p (i n) -> p i n", n=SHARD),
                    in_=wkt[i2 * 256:(i2 + 1) * 256, :]
                        .rearrange("(i p) n -> p i n", p=128))
                nc.gpsimd.dma_start(
                    out=wv_t.rearrange("p (i n) -> p i n", n=SHARD),
                    in_=wvt[i2 * 256:(i2 + 1) * 256, :]
                        .rearrange("(i p) n -> p i n", p=128))
                for u in range(2):
                    i = 2 * i2 + u
                    nc.tensor.matmul(ps_k, lhsT=seqt_sb[:, i:i + 1],
                                     rhs=wk_t[:, u * SHARD:(u + 1) * SHARD],
                                     start=(i == 0), stop=False)
                    nc.tensor.matmul(ps_v, lhsT=seqt_sb[:, i:i + 1],
                                     rhs=wv_t[:, u * SHARD:(u + 1) * SHARD],
                                     start=(i == 0), stop=False)
            nc.tensor.matmul(ps_k, lhsT=ones[0:1, 0:1], rhs=bk_sb,
                             start=False, stop=True)
            nc.tensor.matmul(ps_v, lhsT=ones[0:1, 0:1], rhs=bv_sb,
                             start=False, stop=True)

            # stage finished q/k/v rows in SBUF (DMA cannot read PSUM);
            # bk_sb/bv_sb are dead after the bias matmuls, reuse them.
            nc.vector.tensor_copy(out=stage_sb, in_=ps_q)
            nc.vector.tensor_copy(out=bk_sb, in_=ps_k)
            nc.vector.tensor_copy(out=bv_sb, in_=ps_v)
            nc.sync.dma_start(out=k_s[:, :], in_=bk_sb)
            nc.sync.dma_start(out=v_s[:, :], in_=bv_sb)

            # broadcast q across all 128 partitions via a DRAM bounce.
            # On the scalar queue (right after the wq loads) so it isn't
            # stuck behind the weight streams.
            q_dram = dram.tile([1, SHARD], F32)
            nc.scalar.dma_start(out=q_dram[:, :], in_=stage_sb)
            for s in range(CPT):
                nc.scalar.dma_start(
                    out=q_bcast[:, s * SHARD:(s + 1) * SHARD],
                    in_=q_dram[0:1, :].to_broadcast((128, SHARD)))

            if stop_after == "qkv":
                nc.vector.memset(attn_sb, 0.0)
                nc.sync.dma_start(out=out_s[:, :], in_=attn_sb[0:1, :])
                return nc

            # ---------- phase 2: streamed scores -> exp -> probs@V ----------
            # chunk j (128 keys): DVE computes q.k per head into scores cols
            # [4j:4j+4]; ACT exps them into probs; PE accumulates probs.T @ V
            # into (4,512) and probs.T @ ones into (4,1) denominators.
            ps_attn = psA.tile([LOCAL_HEADS, SHARD], F32)
            ps_d = psA.tile([LOCAL_HEADS, 1], F32)
            for t in range(N_KVT):
                k_t = kp.tile([128, CPT * SHARD], F32, tag="k")
                v_t = vp.tile([128, CPT * SHARD], F32, tag="v")
                r0 = t * CPT * 128
                if t < N_KVT - 1:
                    nc.scalar.dma_start(
                        out=k_t.rearrange("p (s n) -> p s n", n=SHARD),
                        in_=kc[r0:r0 + CPT * 128, :]
                            .rearrange("(s p) n -> p s n", p=128))
                    nc.gpsimd.dma_start(
                        out=v_t.rearrange("p (s n) -> p s n", n=SHARD),
                        in_=vc[r0:r0 + CPT * 128, :]
                            .rearrange("(s p) n -> p s n", p=128))
                else:
                    # chunks 60-62 full; chunk 63 has 127 cached keys + new row
                    nc.scalar.dma_start(
                        out=k_t[:, 0:(CPT - 1) * SHARD]
                            .rearrange("p (s n) -> p s n", n=SHARD),
                        in_=kc[r0:r0 + (CPT - 1) * 128, :]
                            .rearrange("(s p) n -> p s n", p=128))
                    nc.scalar.dma_start(
                        out=k_t[0:127, (CPT - 1) * SHARD:],
                        in_=kc[r0 + (CPT - 1) * 128:KV_LEN, :])
                    nc.sync.dma_start(out=k_t[127:128, (CPT - 1) * SHARD:],
                                      in_=bk_sb)
                    nc.gpsimd.dma_start(
                        out=v_t[:, 0:(CPT - 1) * SHARD]
                            .rearrange("p (s n) -> p s n", n=SHARD),
                        in_=vc[r0:r0 + (CPT - 1) * 128, :]
                            .rearrange("(s p) n -> p s n", p=128))
                    nc.gpsimd.dma_start(
                        out=v_t[0:127, (CPT - 1) * SHARD:],
                        in_=vc[r0 + (CPT - 1) * 128:KV_LEN, :])
                    nc.sync.dma_start(out=v_t[127:128, (CPT - 1) * SHARD:],
                                      in_=bv_sb)
                # one wide multiply + one segmented reduce + one exp per tile
                prod = scr.tile([128, CPT * SHARD], F32, tag="prod")
                nc.vector.tensor_mul(prod, k_t, q_bcast)
                nc.vector.tensor_reduce(
                    out=scores_sb[:, CPT * LOCAL_HEADS * t:
                                  CPT * LOCAL_HEADS * (t + 1)],
                    in_=prod.rearrange("p (c d) -> p c d", d=HEAD),
                    axis=mybir.AxisListType.X,
                    op=mybir.AluOpType.add)
                nc.scalar.activation(
                    out=probs_sb[:, CPT * LOCAL_HEADS * t:
                                 CPT * LOCAL_HEADS * (t + 1)],
                    in_=scores_sb[:, CPT * LOCAL_HEADS * t:
                                  CPT * LOCAL_HEADS * (t + 1)],
                    func=mybir.ActivationFunctionType.Exp,
                    scale=ISQRT_D)
                for s in range(CPT):
                    j = t * CPT + s
                    nc.tensor.matmul(ps_attn,
                                     lhsT=probs_sb[:, 4 * j:4 * j + 4],
                                     rhs=v_t[:, s * SHARD:(s + 1) * SHARD],
                                     start=(j == 0), stop=(j == N_CHUNK - 1))
                    nc.tensor.matmul(ps_d,
                                     lhsT=probs_sb[:, 4 * j:4 * j + 4],
                                     rhs=ones,
                                     start=(j == 0), stop=(j == N_CHUNK - 1))

            nc.vector.reciprocal(out=recip_sb, in_=ps_d)
            nc.vector.tensor_scalar_mul(attn_sb, ps_attn, recip_sb)

            if stop_after == "av":
                nc.sync.dma_start(out=out_s[:, :], in_=attn_sb[0:1, :])
                return nc

            # ---------- phase 3: AllGather attention shards ----------
            cc_in = dram.tile([SHARD], F32)
            cc_out = dram.tile([EMBED], F32, addr_space="Shared")
            for h in range(LOCAL_HEADS):
                nc.sync.dma_start(
                    out=cc_in[h * HEAD:(h + 1) * HEAD],
                    in_=attn_sb[h:h + 1, h * HEAD:(h + 1) * HEAD])
            nc.gpsimd.collective_compute(
                "AllGather", mybir.AluOpType.bypass,
                replica_groups=[list(range(N_CORES))],
                ins=[cc_in.opt()], outs=[cc_out.opt()])
            nc.sync.dma_start(out=attnt_sb,
                              in_=cc_out[:].rearrange("(i p) -> p i", p=128))
            if stop_after == "cc":
                nc.sync.dma_start(out=out_s[:, :], in_=attn_sb[0:1, :])
                return nc

            # ---------- phase 4: output projection (resident Wq^T) ----------
            ps_o = psA.tile([1, SHARD], F32)
            for i in range(N_ICH):
                nc.tensor.matmul(ps_o, lhsT=attnt_sb[:, i:i + 1],
                                 rhs=wq_sb[:, i * SHARD:(i + 1) * SHARD],
                                 start=(i == 0), stop=False)
            nc.tensor.matmul(ps_o, lhsT=ones[0:1, 0:1], rhs=bq_sb,
                             start=False, stop=True)
            nc.vector.tensor_copy(out=stage_sb, in_=ps_o)
            nc.sync.dma_start(out=out_s[:, :], in_=stage_sb)

    return nc


_NC = None


def _get_nc():
    global _NC
    if _NC is None:
        _NC = _build_program()
    return _NC


def _make_in_maps(seq, k_cached, v_cached, Wq_w, Wq_b, Wk_w, Wk_b, Wv_w, Wv_b):
    f = lambda a: np.ascontiguousarray(np.asarray(a, dtype=np.float32))
    seq, k_cached, v_cached = f(seq), f(k_cached), f(v_cached)
    Wq_w, Wk_w, Wv_w = f(Wq_w), f(Wk_w), f(Wv_w)
    Wq_b, Wk_b, Wv_b = f(Wq_b), f(Wk_b), f(Wv_b)

    seqt = np.ascontiguousarray(seq.reshape(N_ICH, 128).T)      # (128, 32)

    in_maps = []
    for c in range(N_CORES):
        sl = slice(c * SHARD, (c + 1) * SHARD)
        in_maps.append({
            "seqt": seqt,
            "wqt": np.ascontiguousarray(Wq_w[sl, :].T),
            "wkt": np.ascontiguousarray(Wk_w[sl, :].T),
            "wvt": np.ascontiguousarray(Wv_w[sl, :].T),
            "b3": np.ascontiguousarray(
                np.stack([Wq_b[sl], Wk_b[sl], Wv_b[sl]])),
            "kc": np.ascontiguousarray(k_cached[:, sl]),
            "vc": np.ascontiguousarray(v_cached[:, sl]),
        })
    return in_maps, (seq, k_cached, v_cached)


def _assemble(results, k_cached, v_cached):
    out = np.concatenate([results[c]["out_s"] for c in range(N_CORES)], axis=1)
    k_row = np.concatenate([results[c]["k_s"] for c in range(N_CORES)], axis=1)
    v_row = np.concatenate([results[c]["v_s"] for c in range(N_CORES)], axis=1)
    k_new = np.concatenate([k_cached, k_row], axis=0)
    v_new = np.concatenate([v_cached, v_row], axis=0)
    return out, k_new, v_new


def kernel(seq, k_cached, v_cached, Wq_w, Wq_b, Wk_w, Wk_b, Wv_w, Wv_b,
           _trace=False):
    nc = _get_nc()
    in_maps, (seq, k_cached, v_cached) = _make_in_maps(
        seq, k_cached, v_cached, Wq_w, Wq_b, Wk_w, Wk_b, Wv_w, Wv_b)
    res = run_bass_kernel_spmd(nc, in_maps, list(range(N_CORES)),
                               trace=_trace)
    outs = _assemble(res.results, k_cached, v_cached)
    if _trace:
        return outs, res
    return outs
